# revision 12
# baseline (speedup 1.0000x reference)
"""DeepseekMoE block (attention + top-2 routed MoE + shared expert) on 8 TRN2
NeuronCores, data-parallel over the batch dimension (B=8 -> one batch per core).

Layout strategy per core (L=1024 tokens, H=1024 hidden):
  - Activations live in "F-layout" [feature-on-partitions, tokens-on-free] so
    every matmul chains without transposes (weights are pre-transposed on host
    to [K_in, M_out]).
  - Per-token scalars (rms scales, softmax 1/Z, gate weights, output gate) are
    produced as [1, L] rows and broadcast across partitions with K=1 rank-1
    matmuls on the TensorEngine.
  - Attention is computed transposed (attT[k, q]) so the key-padding mask and
    exp() fold into one scalar-engine activation (bias = per-partition mask
    column), and ctx comes out of the pT@V matmul directly in F-layout.
  - Precision tiers: float32r (fp32 truncated to fp22, full matmul speed) for
    the attention chain, exact fp32 for the router logits (top-2 selection is
    chaotically sensitive), bf16 for the expert FFNs.
"""

import numpy as np
import ml_dtypes
from contextlib import ExitStack

import concourse.bass as bass
import concourse.mybir as mybir
import concourse.tile as tile
from concourse import bacc
from concourse.bass_utils import run_bass_kernel_spmd
from concourse.masks import make_identity

B, L, H = 8, 1024, 1024
E, I, NH, HD = 8, 256, 4, 256
ISZ = 512
P = 128
NT = L // P      # token blocks
KH = H // P      # hidden slabs
ND = HD // P     # d-blocks per head (=2)
EPS = 1e-6
NEG = -30000.0
INV_SQRT_HD = float(1.0 / np.sqrt(HD))

DT = mybir.dt
F32, BF16, I32 = DT.float32, DT.bfloat16, DT.int32
F32R = DT.float32r
Alu = mybir.AluOpType
Act = mybir.ActivationFunctionType
AX = mybir.AxisListType


def r32(ap):
    """View an fp32 AP as float32r (fp22-truncated matmul operand)."""
    return ap.bitcast(DT.float32r)


def build():
    nc = bacc.Bacc("TRN2", target_bir_lowering=False, debug=False)

    def din(name, shape, dt):
        return nc.dram_tensor(name, shape, dt, kind="ExternalInput").ap()

    xT = din("x_t", [H, L], F32)
    tcc = din("tc_col", [P, 1], F32)
    wqk = din("wqkT", [H, 2 * H], F32R)
    wvm = din("wvT", [H, H], F32R)
    wom = din("woT", [H, H], F32R)
    wgm = din("wgT", [H, E * I], BF16)
    wum = din("wuT", [H, E * I], BF16)
    wdm = din("wdT", [E * I + ISZ, H], BF16)
    wsg = din("wsgT", [H, ISZ], BF16)
    wsu = din("wsuT", [H, ISZ], BF16)
    wgt = din("wgateT", [H, E], F32)
    ogm = din("ogc", [P, KH], BF16)
    ogb = din("ogb", [1, 1], F32)
    bqk = din("bqk", [P, 16], F32)
    bvr = din("bv_row", [1, H], F32R)
    bop = din("bop", [P, KH], F32)
    outm = nc.dram_tensor("out", [H, L], F32, kind="ExternalOutput").ap()

    with tile.TileContext(nc) as tc:
        es = {}  # manually closed pools

        def open_pool(key, **kw):
            st = ExitStack()
            pool = st.enter_context(tc.tile_pool(name=key, **kw))
            es[key] = st
            return pool

        with ExitStack() as top:
            const = top.enter_context(tc.tile_pool(name="const", bufs=1))

            ident = const.tile([P, P], F32, name="ident")
            make_identity(nc, ident)
            ones_cb = const.tile([P, 1], BF16, name="ones_cb")
            nc.gpsimd.memset(ones_cb[:], 1.0)
            ones_cf = const.tile([P, 1], F32, name="ones_cf")
            nc.gpsimd.memset(ones_cf[:], 1.0)
            ones_row_f = const.tile([1, P], F32, name="ones_row_f")
            nc.gpsimd.memset(ones_row_f[:], 1.0)
            ones_row = const.tile([1, P], F32R, name="ones_row")
            nc.scalar.copy(ones_row[:], ones_row_f[:])
            eps_col = const.tile([P, 1], F32, name="eps_col")
            nc.gpsimd.memset(eps_col[:], EPS)
            tc_sb = const.tile([P, 1], F32, name="tc_sb")
            nc.sync.dma_start(tc_sb[:], tcc[:, :])

            # key-padding masks: maskc[:, kb] = 0 if (kb*128+p) < tc else NEG
            iog = const.tile([P, NT], I32, name="iog")
            nc.gpsimd.iota(iog[:], pattern=[[P, NT]], base=0, channel_multiplier=1)
            iogf = const.tile([P, NT], F32, name="iogf")
            nc.vector.tensor_copy(iogf[:], iog[:])
            mask01 = const.tile([P, NT], F32, name="mask01")
            nc.vector.tensor_scalar(mask01[:], iogf[:], tc_sb[:], None, op0=Alu.is_ge)
            maskc = const.tile([P, NT], F32, name="maskc")
            nc.scalar.mul(maskc[:], mask01[:], NEG)
            # valid[0, n] = 1 if n < tc else 0
            ior = const.tile([1, L], I32, name="ior")
            nc.gpsimd.iota(ior[:], pattern=[[1, L]], base=0, channel_multiplier=0)
            iorf = const.tile([1, L], F32, name="iorf")
            nc.vector.tensor_copy(iorf[:], ior[:])
            valid = const.tile([1, L], F32, name="valid")
            nc.vector.tensor_scalar(valid[:], iorf[:], tc_sb[0:1, :], None, op0=Alu.is_lt)

            bias_p = top.enter_context(tc.tile_pool(name="biasp", bufs=1))
            bqk_sb = bias_p.tile([P, 16], F32, name="bqk")
            nc.sync.dma_start(bqk_sb[:], bqk[:, :])
            bvr_sb = bias_p.tile([1, H], F32R, name="bvr")
            nc.sync.dma_start(bvr_sb[:], bvr[:, :])
            bop_sb = bias_p.tile([P, KH], F32, name="bop")
            nc.sync.dma_start(bop_sb[:], bop[:, :])

            # ---------------- phase A: rms0 + nx ----------------
            nxp = open_pool("nx", bufs=1, side="right")
            NX = [nxp.tile([P, L], F32R, name=f"nx{k}") for k in range(KH)]
            with ExitStack() as ph:
                xp = ph.enter_context(tc.tile_pool(name="xa", bufs=1))
                X = []
                for k in range(KH):
                    t = xp.tile([P, L], F32, name=f"x{k}")
                    nc.sync.dma_start(t[:], xT[k * P:(k + 1) * P, :])
                    X.append(t)
                sq = ph.enter_context(tc.tile_pool(name="sq0", bufs=KH))
                pp = ph.enter_context(tc.tile_pool(name="ps0", bufs=2, space="PSUM"))
                pb = ph.enter_context(tc.tile_pool(name="ps0b", bufs=2, space="PSUM"))
                bc = ph.enter_context(tc.tile_pool(name="bc0", bufs=1))
                xsq = []
                for k in range(KH):
                    t = sq.tile([P, L], BF16, tag="xsq", name="xsq")
                    nc.scalar.activation(t[:], X[k][:], Act.Square)
                    xsq.append(t)
                r0row = bc.tile([1, L], F32, name="r0row")
                sroot = bc.tile([1, L], F32, name="sroot0")
                for j in range(2):
                    ps = pp.tile([1, 512], F32, tag="ss", name="ss")
                    for k in range(KH):
                        nc.tensor.matmul(ps[:], ones_cb[:], xsq[k][:, j * 512:(j + 1) * 512],
                                         start=(k == 0), stop=(k == KH - 1))
                    nc.scalar.activation(sroot[0:1, j * 512:(j + 1) * 512], ps[:],
                                         Act.Sqrt, bias=eps_col[0:1, :], scale=1.0 / H)
                    nc.vector.reciprocal(r0row[0:1, j * 512:(j + 1) * 512],
                                         sroot[0:1, j * 512:(j + 1) * 512])
                r0row_r = bc.tile([1, L], F32R, name="r0row_r")
                nc.scalar.copy(r0row_r[:], r0row[:])
                r0bc = bc.tile([P, L], F32, name="r0bc")
                for j in range(2):
                    psb = pb.tile([P, 512], F32, tag="bc", name="bc")
                    nc.tensor.matmul(psb[:], ones_row[:],
                                     r0row_r[0:1, j * 512:(j + 1) * 512],
                                     start=True, stop=True)
                    nc.scalar.copy(r0bc[:, j * 512:(j + 1) * 512], psb[:])
                for k in range(KH):
                    nc.vector.tensor_mul(NX[k][:], X[k][:], r0bc[:])

            # ---------------- phase B: QKV ----------------
            qkvp = open_pool("qkv", bufs=1)
            Q = [qkvp.tile([P, L], BF16, name=f"q{i}") for i in range(KH)]
            K = [qkvp.tile([P, L], BF16, name=f"k{i}") for i in range(KH)]
            V = [qkvp.tile([P, L], BF16, name=f"v{i}") for i in range(NT)]

            for half, dst in ((0, Q), (1, K)):
                with ExitStack() as ph:
                    wp = ph.enter_context(tc.tile_pool(name=f"wqk{half}", bufs=1))
                    pp = ph.enter_context(tc.tile_pool(name=f"psqk{half}", bufs=4, space="PSUM"))
                    w_sb = []
                    for k in range(KH):
                        t = wp.tile([P, H], F32R, name=f"wqk{half}_{k}")
                        nc.sync.dma_start(t[:], wqk[k * P:(k + 1) * P, half * H:(half + 1) * H])
                        w_sb.append(t)
                    for fb in range(KH):
                        pts = [pp.tile([P, 512], F32, tag="qk", name="qk") for _ in range(2)]
                        for k in range(KH):
                            for j in range(2):
                                nc.tensor.matmul(
                                    pts[j][:],
                                    w_sb[k][:, fb * P:(fb + 1) * P],
                                    NX[k][:, j * 512:(j + 1) * 512],
                                    start=(k == 0), stop=(k == KH - 1))
                        for j in range(2):
                            nc.scalar.activation(dst[fb][:, j * 512:(j + 1) * 512], pts[j][:],
                                                 Act.Identity,
                                                 bias=bqk_sb[:, half * KH + fb:half * KH + fb + 1])

            with ExitStack() as ph:
                wp = ph.enter_context(tc.tile_pool(name="wv", bufs=1))
                pp = ph.enter_context(tc.tile_pool(name="psv", bufs=4, space="PSUM"))
                wv_sb = []
                for k in range(KH):
                    t = wp.tile([P, H], F32R, name=f"wv{k}")
                    nc.sync.dma_start(t[:], wvm[k * P:(k + 1) * P, :])
                    wv_sb.append(t)
                for tb in range(NT):
                    pts = [pp.tile([P, 512], F32, tag="v", name="v") for _ in range(2)]
                    for k in range(KH):
                        for j in range(2):
                            nc.tensor.matmul(
                                pts[j][:],
                                NX[k][:, tb * P:(tb + 1) * P],
                                wv_sb[k][:, j * 512:(j + 1) * 512],
                                start=(k == 0), stop=False)
                    for j in range(2):
                        # homogeneous bias row: out += 1 * bv
                        nc.tensor.matmul(pts[j][:], ones_row[:],
                                         bvr_sb[0:1, j * 512:(j + 1) * 512],
                                         start=False, stop=True)
                        nc.vector.tensor_copy(V[tb][:, j * 512:(j + 1) * 512], pts[j][:])
            es["nx"].close()

            # ---------------- phase C: attention ----------------
            ctxp = open_pool("ctx", bufs=1, side="right")
            CTX = [ctxp.tile([P, L], F32R, name=f"ctx{i}") for i in range(KH)]
            with ExitStack() as ph:
                ptp = ph.enter_context(tc.tile_pool(name="pt", bufs=10))
                zp = ph.enter_context(tc.tile_pool(name="zrow", bufs=2))
                zbp = ph.enter_context(tc.tile_pool(name="zbc", bufs=2))
                pa = ph.enter_context(tc.tile_pool(name="psatt", bufs=2, space="PSUM"))
                pz = ph.enter_context(tc.tile_pool(name="psz", bufs=2, space="PSUM"))
                pc = ph.enter_context(tc.tile_pool(name="psctx", bufs=2, space="PSUM"))
                pbb = ph.enter_context(tc.tile_pool(name="psbcz", bufs=1, space="PSUM"))
                for h in range(NH):
                    pts = []
                    for kb in range(NT):
                        pt_t = ptp.tile([P, L], BF16, tag="pt", name="pt")
                        for qh in range(2):
                            pa_t = pa.tile([P, 512], F32, tag="att", name="att")
                            for t in range(2):
                                nc.tensor.matmul(
                                    pa_t[:],
                                    K[2 * h + t][:, kb * P:(kb + 1) * P],
                                    Q[2 * h + t][:, qh * 512:(qh + 1) * 512],
                                    start=(t == 0), stop=(t == 1))
                            nc.scalar.activation(pt_t[:, qh * 512:(qh + 1) * 512], pa_t[:],
                                                 Act.Exp, bias=maskc[:, kb:kb + 1],
                                                 scale=INV_SQRT_HD)
                        pts.append(pt_t)
                    zrow = zp.tile([1, L], F32, tag="z", name="z")
                    for qh in range(2):
                        pz_t = pz.tile([1, 512], F32, tag="z", name="z")
                        for kb in range(NT):
                            nc.tensor.matmul(pz_t[:], ones_cb[:],
                                             pts[kb][:, qh * 512:(qh + 1) * 512],
                                             start=(kb == 0), stop=(kb == NT - 1))
                        nc.vector.reciprocal(zrow[0:1, qh * 512:(qh + 1) * 512], pz_t[:])
                    zrow_r = zp.tile([1, L], F32R, tag="zr", name="zr")
                    nc.scalar.copy(zrow_r[:], zrow[:])
                    zbc = zbp.tile([P, L], F32, tag="zbc", name="zbc")
                    for qh in range(2):
                        pb_t = pbb.tile([P, 512], F32, tag="bcz", name="bcz")
                        nc.tensor.matmul(pb_t[:], ones_row[:],
                                         zrow_r[0:1, qh * 512:(qh + 1) * 512],
                                         start=True, stop=True)
                        nc.scalar.copy(zbc[:, qh * 512:(qh + 1) * 512], pb_t[:])
                    for db in range(ND):
                        for qh in range(2):
                            pc_t = pc.tile([P, 512], F32, tag="ctx", name="ctx")
                            for kb in range(NT):
                                nc.tensor.matmul(
                                    pc_t[:],
                                    V[kb][:, h * HD + db * P: h * HD + (db + 1) * P],
                                    pts[kb][:, qh * 512:(qh + 1) * 512],
                                    start=(kb == 0), stop=(kb == NT - 1))
                            nc.vector.tensor_mul(
                                CTX[2 * h + db][:, qh * 512:(qh + 1) * 512],
                                pc_t[:], zbc[:, qh * 512:(qh + 1) * 512])
            es["qkv"].close()

            # ---------------- phase D: out_proj + residual ----------------
            x1p = open_pool("x1", bufs=1)
            X1 = [x1p.tile([P, L], F32, name=f"x1_{i}") for i in range(KH)]
            with ExitStack() as ph:
                wp = ph.enter_context(tc.tile_pool(name="wo", bufs=1))
                pp = ph.enter_context(tc.tile_pool(name="pso", bufs=4, space="PSUM"))
                xp2 = ph.enter_context(tc.tile_pool(name="xd", bufs=1))
                X = []
                for k in range(KH):
                    t = xp2.tile([P, L], F32, name=f"xd{k}")
                    nc.sync.dma_start(t[:], xT[k * P:(k + 1) * P, :])
                    X.append(t)
                wo_sb = []
                for k in range(KH):
                    t = wp.tile([P, H], F32R, name=f"wo{k}")
                    nc.sync.dma_start(t[:], wom[k * P:(k + 1) * P, :])
                    wo_sb.append(t)
                for fb in range(KH):
                    pts = [pp.tile([P, 512], F32, tag="o", name="o") for _ in range(2)]
                    for k in range(KH):
                        for j in range(2):
                            nc.tensor.matmul(
                                pts[j][:],
                                wo_sb[k][:, fb * P:(fb + 1) * P],
                                CTX[k][:, j * 512:(j + 1) * 512],
                                start=(k == 0), stop=(k == KH - 1))
                    for j in range(2):
                        nc.vector.scalar_tensor_tensor(
                            X1[fb][:, j * 512:(j + 1) * 512],
                            pts[j][:], bop_sb[:, fb:fb + 1],
                            X[fb][:, j * 512:(j + 1) * 512],
                            op0=Alu.add, op1=Alu.add)
            es["ctx"].close()

            # ---------------- phase E: rms1 + xhat + r_cols ----------------
            xhp = open_pool("xhat", bufs=1, side="right")
            XH = [xhp.tile([P, L], BF16, name=f"xh{k}") for k in range(KH)]
            r_cols = xhp.tile([P, NT], F32, name="r_cols")
            with ExitStack() as ph:
                sq = ph.enter_context(tc.tile_pool(name="sq1", bufs=KH))
                pp = ph.enter_context(tc.tile_pool(name="ps1", bufs=2, space="PSUM"))
                pb = ph.enter_context(tc.tile_pool(name="ps1b", bufs=2, space="PSUM"))
                ptr = ph.enter_context(tc.tile_pool(name="ps1t", bufs=1, space="PSUM"))
                bc = ph.enter_context(tc.tile_pool(name="bc1", bufs=1))
                xsq = []
                for k in range(KH):
                    t = sq.tile([P, L], BF16, tag="x1sq", name="x1sq")
                    nc.scalar.activation(t[:], X1[k][:], Act.Square)
                    xsq.append(t)
                rrow = bc.tile([1, L], F32, name="rrow")
                sroot = bc.tile([1, L], F32, name="sroot1")
                for j in range(2):
                    ps = pp.tile([1, 512], F32, tag="ss", name="ss")
                    for k in range(KH):
                        nc.tensor.matmul(ps[:], ones_cb[:], xsq[k][:, j * 512:(j + 1) * 512],
                                         start=(k == 0), stop=(k == KH - 1))
                    nc.scalar.activation(sroot[0:1, j * 512:(j + 1) * 512], ps[:],
                                         Act.Sqrt, bias=eps_col[0:1, :], scale=1.0 / H)
                    nc.vector.reciprocal(rrow[0:1, j * 512:(j + 1) * 512],
                                         sroot[0:1, j * 512:(j + 1) * 512])
                rrow_r = bc.tile([1, L], F32R, name="rrow_r")
                nc.scalar.copy(rrow_r[:], rrow[:])
                rbc = bc.tile([P, L], F32, name="rbc")
                for j in range(2):
                    psb = pb.tile([P, 512], F32, tag="bc", name="bc")
                    nc.tensor.matmul(psb[:], ones_row[:],
                                     rrow_r[0:1, j * 512:(j + 1) * 512],
                                     start=True, stop=True)
                    nc.scalar.copy(rbc[:, j * 512:(j + 1) * 512], psb[:])
                for k in range(KH):
                    nc.vector.tensor_mul(XH[k][:], X1[k][:], rbc[:])
                # r as per-token columns [128, NT] via tiny transposes
                ptt = ptr.tile([P, NT], F32, tag="rt", name="rt")
                for tb in range(NT):
                    nc.tensor.transpose(ptt[:, tb:tb + 1],
                                        rrow[0:1, tb * P:(tb + 1) * P],
                                        ident[0:1, 0:1])
                nc.scalar.copy(r_cols[:], ptt[:])

            # ---------------- phase F: router gating ----------------
            wbcp = open_pool("wbc", bufs=1, side="right")
            WBC = [wbcp.tile([P, L], BF16, name=f"wbc{e}") for e in range(E)]
            with ExitStack() as ph:
                wp = ph.enter_context(tc.tile_pool(name="wgate", bufs=1))
                gp = ph.enter_context(tc.tile_pool(name="gating", bufs=4))
                pg = ph.enter_context(tc.tile_pool(name="psg", bufs=2, space="PSUM"))
                pt_ = ph.enter_context(tc.tile_pool(name="psgt", bufs=2, space="PSUM"))
                pwb = ph.enter_context(tc.tile_pool(name="pswb", bufs=2, space="PSUM"))
                wrows = xhp.tile([E, L], F32R, name="wrows")
                wgt_sb = []
                for k in range(KH):
                    t = wp.tile([P, E], F32, name=f"wgt{k}")
                    nc.sync.dma_start(t[:], wgt[k * P:(k + 1) * P, :])
                    wgt_sb.append(t)
                for tb in range(NT):
                    pg_t = pg.tile([P, E], F32, tag="g", name="g")
                    for k in range(KH):
                        nc.tensor.matmul(pg_t[:], X1[k][:, tb * P:(tb + 1) * P], wgt_sb[k][:],
                                         start=(k == 0), stop=(k == KH - 1))
                    s_t = gp.tile([P, E], F32, tag="s", name="s")
                    nc.scalar.activation(s_t[:], pg_t[:], Act.Exp,
                                         scale=r_cols[:, tb:tb + 1])
                    m1 = gp.tile([P, 1], F32, tag="m1", name="m1")
                    nc.vector.reduce_max(m1[:], s_t[:], axis=AX.X)
                    ml = gp.tile([P, E], F32, tag="ml", name="ml")
                    nc.vector.tensor_scalar(ml[:], s_t[:], m1[:], None, op0=Alu.is_lt)
                    s2 = gp.tile([P, E], F32, tag="s2", name="s2")
                    nc.vector.tensor_mul(s2[:], s_t[:], ml[:])
                    m2 = gp.tile([P, 1], F32, tag="m2", name="m2")
                    nc.vector.reduce_max(m2[:], s2[:], axis=AX.X)
                    keep = gp.tile([P, E], F32, tag="keep", name="keep")
                    nc.vector.tensor_scalar(keep[:], s_t[:], m2[:], None, op0=Alu.is_ge)
                    ssum = gp.tile([P, 1], F32, tag="ssum", name="ssum")
                    nc.vector.tensor_add(ssum[:], m1[:], m2[:])
                    srec = gp.tile([P, 1], F32, tag="srec", name="srec")
                    nc.vector.reciprocal(srec[:], ssum[:])
                    wt = gp.tile([P, E], F32, tag="wt", name="wt")
                    nc.vector.scalar_tensor_tensor(wt[:], s_t[:], srec[:], keep[:],
                                                   op0=Alu.mult, op1=Alu.mult)
                    pt_t = pt_.tile([E, P], F32, tag="wtT", name="wtT")
                    nc.tensor.transpose(pt_t[:], wt[:], ident[:])
                    nc.scalar.copy(wrows[:, tb * P:(tb + 1) * P], pt_t[:])
                wrow_e = []
                for e in range(E):
                    t = xhp.tile([1, L], F32R, name=f"wrow{e}")
                    nc.sync.dma_start(t[:], wrows[e:e + 1, :])
                    wrow_e.append(t)
                for e in range(E):
                    for j in range(2):
                        pw_t = pwb.tile([P, 512], F32, tag="wbc", name="wbc")
                        nc.tensor.matmul(pw_t[:], ones_row[:],
                                         wrow_e[e][0:1, j * 512:(j + 1) * 512],
                                         start=True, stop=True)
                        nc.scalar.copy(WBC[e][:, j * 512:(j + 1) * 512], pw_t[:])
            es["x1"].close()

            # ---------------- phase G/H: expert gate/up (routed + shared) ----------------
            ap_ = open_pool("acts", bufs=1)
            A = [ap_.tile([P, L], BF16, name=f"a{i}") for i in range(2 * E)]
            ASH = [ap_.tile([P, L], BF16, name=f"ash{i}") for i in range(ISZ // P)]
            with ExitStack() as ph:
                wp = ph.enter_context(tc.tile_pool(name="wgu", bufs=1))
                tmp = ph.enter_context(tc.tile_pool(name="tmpgu", bufs=2))
                pp = ph.enter_context(tc.tile_pool(name="psgu", bufs=4, space="PSUM"))
                wg_sb, wu_sb = [], []
                for k in range(KH):
                    t = wp.tile([P, E * I], BF16, name=f"wg{k}")
                    nc.sync.dma_start(t[:], wgm[k * P:(k + 1) * P, :])
                    wg_sb.append(t)
                    t = wp.tile([P, E * I], BF16, name=f"wu{k}")
                    nc.sync.dma_start(t[:], wum[k * P:(k + 1) * P, :])
                    wu_sb.append(t)
                for fb in range(2 * E):
                    e = fb // 2
                    pg_ = [pp.tile([P, 512], F32, tag="gu", name="gu") for _ in range(2)]
                    for k in range(KH):
                        for j in range(2):
                            nc.tensor.matmul(pg_[j][:], wg_sb[k][:, fb * P:(fb + 1) * P],
                                             XH[k][:, j * 512:(j + 1) * 512],
                                             start=(k == 0), stop=(k == KH - 1))
                    sgm = tmp.tile([P, L], BF16, tag="sgm", name="sgm")
                    for j in range(2):
                        nc.scalar.activation(sgm[:, j * 512:(j + 1) * 512], pg_[j][:], Act.Sigmoid)
                    sg = tmp.tile([P, L], BF16, tag="sg", name="sg")
                    for j in range(2):
                        nc.vector.tensor_mul(sg[:, j * 512:(j + 1) * 512], pg_[j][:],
                                             sgm[:, j * 512:(j + 1) * 512])
                    pu_ = [pp.tile([P, 512], F32, tag="gu", name="gu") for _ in range(2)]
                    for k in range(KH):
                        for j in range(2):
                            nc.tensor.matmul(pu_[j][:], wu_sb[k][:, fb * P:(fb + 1) * P],
                                             XH[k][:, j * 512:(j + 1) * 512],
                                             start=(k == 0), stop=(k == KH - 1))
                    ta = tmp.tile([P, L], BF16, tag="ta", name="ta")
                    for j in range(2):
                        nc.vector.tensor_mul(ta[:, j * 512:(j + 1) * 512], pu_[j][:],
                                             sg[:, j * 512:(j + 1) * 512])
                    nc.vector.tensor_mul(A[fb][:], ta[:], WBC[e][:])
            es["wbc"].close()

            with ExitStack() as ph:
                wp = ph.enter_context(tc.tile_pool(name="wsgu", bufs=1))
                tmp = ph.enter_context(tc.tile_pool(name="tmpsgu", bufs=2))
                pp = ph.enter_context(tc.tile_pool(name="pssgu", bufs=4, space="PSUM"))
                wsg_sb, wsu_sb = [], []
                for k in range(KH):
                    t = wp.tile([P, ISZ], BF16, name=f"wsg{k}")
                    nc.sync.dma_start(t[:], wsg[k * P:(k + 1) * P, :])
                    wsg_sb.append(t)
                    t = wp.tile([P, ISZ], BF16, name=f"wsu{k}")
                    nc.sync.dma_start(t[:], wsu[k * P:(k + 1) * P, :])
                    wsu_sb.append(t)
                for fb in range(ISZ // P):
                    pg_ = [pp.tile([P, 512], F32, tag="sgu", name="sgu") for _ in range(2)]
                    for k in range(KH):
                        for j in range(2):
                            nc.tensor.matmul(pg_[j][:], wsg_sb[k][:, fb * P:(fb + 1) * P],
                                             XH[k][:, j * 512:(j + 1) * 512],
                                             start=(k == 0), stop=(k == KH - 1))
                    sgm = tmp.tile([P, L], BF16, tag="ssgm", name="ssgm")
                    for j in range(2):
                        nc.scalar.activation(sgm[:, j * 512:(j + 1) * 512], pg_[j][:], Act.Sigmoid)
                    sg = tmp.tile([P, L], BF16, tag="ssg", name="ssg")
                    for j in range(2):
                        nc.vector.tensor_mul(sg[:, j * 512:(j + 1) * 512], pg_[j][:],
                                             sgm[:, j * 512:(j + 1) * 512])
                    pu_ = [pp.tile([P, 512], F32, tag="sgu", name="sgu") for _ in range(2)]
                    for k in range(KH):
                        for j in range(2):
                            nc.tensor.matmul(pu_[j][:], wsu_sb[k][:, fb * P:(fb + 1) * P],
                                             XH[k][:, j * 512:(j + 1) * 512],
                                             start=(k == 0), stop=(k == KH - 1))
                    for j in range(2):
                        nc.vector.tensor_mul(ASH[fb][:, j * 512:(j + 1) * 512], pu_[j][:],
                                             sg[:, j * 512:(j + 1) * 512])
            es["xhat"].close()

            # ---------------- phase I: down proj (routed + shared fused) ----------------
            yp = open_pool("y", bufs=1, side="right")
            Y = [yp.tile([P, L], F32, name=f"y{i}") for i in range(KH)]
            YB = [yp.tile([P, L], BF16, name=f"yb{i}") for i in range(KH)]
            AALL = A + ASH
            NKD = len(AALL)  # 20
            with ExitStack() as ph:
                wp = ph.enter_context(tc.tile_pool(name="wd", bufs=1))
                pp = ph.enter_context(tc.tile_pool(name="psd", bufs=4, space="PSUM"))
                wd_sb = []
                for k in range(NKD):
                    t = wp.tile([P, H], BF16, name=f"wd{k}")
                    nc.sync.dma_start(t[:], wdm[k * P:(k + 1) * P, :])
                    wd_sb.append(t)
                for hb in range(KH):
                    pts = [pp.tile([P, 512], F32, tag="y", name="y") for _ in range(2)]
                    for k in range(NKD):
                        for j in range(2):
                            nc.tensor.matmul(pts[j][:], wd_sb[k][:, hb * P:(hb + 1) * P],
                                             AALL[k][:, j * 512:(j + 1) * 512],
                                             start=(k == 0), stop=(k == NKD - 1))
                    for j in range(2):
                        nc.scalar.copy(Y[hb][:, j * 512:(j + 1) * 512], pts[j][:])
                        nc.vector.tensor_copy(YB[hb][:, j * 512:(j + 1) * 512], pts[j][:])
            es["acts"].close()

            # ---------------- phase J: output gate + final mask ----------------
            with ExitStack() as ph:
                wp = ph.enter_context(tc.tile_pool(name="wog", bufs=1))
                fr = ph.enter_context(tc.tile_pool(name="final", bufs=1))
                op_ = ph.enter_context(tc.tile_pool(name="outp", bufs=3))
                pg = ph.enter_context(tc.tile_pool(name="psog", bufs=2, space="PSUM"))
                pbf = ph.enter_context(tc.tile_pool(name="psfin", bufs=1, space="PSUM"))
                ogc_sb = wp.tile([P, KH], BF16, name="ogc")
                nc.sync.dma_start(ogc_sb[:], ogm[:, :])
                ogb_sb = wp.tile([1, 1], F32, name="ogb")
                nc.sync.dma_start(ogb_sb[:], ogb[:, :])
                sigrow = fr.tile([1, L], F32, name="sigrow")
                for j in range(2):
                    pg_t = pg.tile([1, 512], F32, tag="og", name="og")
                    for k in range(KH):
                        nc.tensor.matmul(pg_t[:], ogc_sb[:, k:k + 1],
                                         YB[k][:, j * 512:(j + 1) * 512],
                                         start=(k == 0), stop=(k == KH - 1))
                    nc.scalar.activation(sigrow[0:1, j * 512:(j + 1) * 512], pg_t[:],
                                         Act.Sigmoid, bias=ogb_sb[0:1, :])
                svrow = fr.tile([1, L], F32R, name="svrow")
                nc.vector.tensor_mul(svrow[:], sigrow[:], valid[:])
                svb = fr.tile([P, L], F32, name="svb")
                for j in range(2):
                    pb_t = pbf.tile([P, 512], F32, tag="fin", name="fin")
                    nc.tensor.matmul(pb_t[:], ones_row[:],
                                     svrow[0:1, j * 512:(j + 1) * 512],
                                     start=True, stop=True)
                    nc.scalar.copy(svb[:, j * 512:(j + 1) * 512], pb_t[:])
                for hb in range(KH):
                    ot = op_.tile([P, L], F32, tag="ot", name="ot")
                    nc.vector.tensor_mul(ot[:], Y[hb][:], svb[:])
                    nc.sync.dma_start(outm[hb * P:(hb + 1) * P, :], ot[:])
            es["y"].close()

    nc.compile()
    return nc


_CACHE = {}


def _get_program():
    if "nc" not in _CACHE:
        _CACHE["nc"] = build()
    return _CACHE["nc"]


def _prep_inputs(inputs):
    f32 = np.float32
    bf = ml_dtypes.bfloat16
    g = lambda k: np.asarray(inputs[k]).astype(f32)

    hs = g("hidden_states")
    tcs = np.asarray(inputs["true_counts"]).astype(np.int64).reshape(B)
    cnw, gnw, snw = g("context_norm_w"), g("gate_norm_w"), g("shared_norm_w")
    ipw, ipb = g("in_proj_w"), g("in_proj_b")
    opw, opb = g("out_proj_w"), g("out_proj_b")
    gw = g("gate_w")
    enw = g("expert_norm_w")
    egw, euw, edw = g("expert_gate_w"), g("expert_up_w"), g("expert_down_w")
    sgw, suw, sdw = g("shared_gate_w"), g("shared_up_w"), g("shared_down_w")
    ogw, ogb_ = g("out_gate_w"), g("out_gate_b")

    shared = {
        "wqkT": np.ascontiguousarray((ipw[:2 * H] * cnw[None, :]).T),
        "wvT": np.ascontiguousarray((ipw[2 * H:] * cnw[None, :]).T),
        "woT": np.ascontiguousarray(opw.T),
        "wgT": np.ascontiguousarray((egw * enw[:, None, :]).reshape(E * I, H).T.astype(bf)),
        "wuT": np.ascontiguousarray((euw * enw[:, None, :]).reshape(E * I, H).T.astype(bf)),
        "wdT": np.ascontiguousarray(np.concatenate(
            [edw.transpose(0, 2, 1).reshape(E * I, H), sdw.T], axis=0).astype(bf)),
        "wsgT": np.ascontiguousarray((sgw * snw[None, :]).T.astype(bf)),
        "wsuT": np.ascontiguousarray((suw * snw[None, :]).T.astype(bf)),
        "wgateT": np.ascontiguousarray((gw * gnw[None, :]).T),
        "ogc": np.ascontiguousarray(ogw.reshape(KH, P).T.astype(bf)),
        "ogb": ogb_.reshape(1, 1),
        "bqk": np.ascontiguousarray(ipb[:2 * H].reshape(16, P).T),
        "bv_row": np.ascontiguousarray(ipb[2 * H:].reshape(1, H)),
        "bop": np.ascontiguousarray(opb.reshape(KH, P).T),
    }
    in_maps = []
    for b in range(B):
        m = dict(shared)
        m["x_t"] = np.ascontiguousarray(hs[b].T)
        m["tc_col"] = np.full((P, 1), float(tcs[b]), f32)
        in_maps.append(m)
    return in_maps


LAST_RESULT = None


def _run(inputs, **kw):
    global LAST_RESULT
    nc = _get_program()
    in_maps = _prep_inputs(inputs)
    res = run_bass_kernel_spmd(nc, in_maps, core_ids=list(range(B)), **kw)
    LAST_RESULT = res
    out = np.stack([res.results[b]["out"].T for b in range(B)])
    return np.ascontiguousarray(out.astype(np.float32))


def kernel(**inputs):
    return _run(inputs)
